# revision 24
# baseline (speedup 1.0000x reference)
"""CSR Linear kernel for TRN2: out = x @ W^T + bias, W from COO nonzeros.

Strategy: data-parallel over tokens across 8 NeuronCores. Host densifies the
sparse weight into A[in, out] (duplicate coords summed) in bf16; each core
computes its 1024-token shard as out^T = A^T-tiles stationary on the PE with
x^T streaming:  psum[128 outf, 512 tok] += A_tile[128 k, 128 outf].T @
xT[128 k, 512 tok].  With out-features on PSUM partitions the bias add is a
per-partition tensor_scalar on the eviction. bf16 operands halve DMA bytes
and enable fast weight load; phase A runs the first 4 out-tiles k-outer so
DMA demand stays under the per-core HBM rate from the first matmul, phase B
runs o-major k-sweeps at pure PE rate with per-sweep evictions.
"""

import os
import sys
import types

import ml_dtypes
import numpy as np

TOKENS = 8192
IN_F = 4096
OUT_F = 4096
N_CORES = 8
P = 128

_CACHE = {}


def _ensure_ntff_hook():
    """Register the axon NTFF profile hook if the antenv stub lacks it.

    Only needed when tracing (BASS_TRACE=1); harmless otherwise. In
    environments with a real antenv.axon_hooks this is a no-op.
    """
    try:
        import antenv.axon_hooks  # noqa: F401

        return
    except ImportError:
        pass
    try:
        import antenv
        from trn_agent_boot.trn_boot import _ntff_profile_via_ctypes

        hooks = types.ModuleType("antenv.axon_hooks")
        hooks._hook = _ntff_profile_via_ctypes("/opt/axon/libaxon_pjrt.so")
        hooks.set_axon_ntff_profile_hook = lambda h: setattr(hooks, "_hook", h)
        hooks.get_axon_ntff_profile_hook = lambda: hooks._hook
        sys.modules["antenv.axon_hooks"] = hooks
        antenv.axon_hooks = hooks
    except Exception:
        pass


def _patch_upload():
    """Make trace artifact upload fall back to the local tmpdir when no
    artifact bucket is reachable (container environments)."""
    from concourse import bass_utils

    orig = bass_utils.upload_artifacts
    if getattr(orig, "_kernel_patched", False):
        return

    def _safe_upload(tmpdir):
        try:
            return orig(tmpdir)
        except Exception:
            return tmpdir

    _safe_upload._kernel_patched = True
    bass_utils.upload_artifacts = _safe_upload


def build_program(tok_per_core=TOKENS // N_CORES, in_f=IN_F, out_f=OUT_F):
    """Build + compile the per-core Bass program.

    outT[out_f, tok_per_core] = sum_k A[k, :].T-tiles @ xT[k, tokens] + bias
    with A [in_f, out_f] bf16 (host-densified W^T), xT [in_f, tok] bf16.
    """
    key = (tok_per_core, in_f, out_f)
    if key in _CACHE:
        return _CACHE[key]

    import concourse.bacc as bacc
    import concourse.mybir as mybir
    import concourse.tile as tile

    KO = in_f // P  # 32 contraction tiles
    NB = out_f // P  # 32 out-feature tiles
    NH = tok_per_core // 512  # 2 token halves (psum bank = 512 f32)
    A_TILES = 4  # phase-A out-tiles (k-outer), 4*NH = 8 psum banks

    nc = bacc.Bacc("TRN2", target_bir_lowering=False, debug=False)

    # xt2[p, ko*T + t] = x_shard^T[ko*128+p, t]
    xt = nc.dram_tensor("xt", [P, KO * tok_per_core], mybir.dt.bfloat16, kind="ExternalInput")
    # wt2[nb*128+p, ko*128+o] = A[ko*128+p, nb*128+o]
    wt = nc.dram_tensor("wt", [out_f, in_f], mybir.dt.bfloat16, kind="ExternalInput")
    # biasr[p, nb] = bias[nb*128+p]
    biasr = nc.dram_tensor("biasr", [P, NB], mybir.dt.float32, kind="ExternalInput")
    # outT[nb*128+p, t] = out[t, nb*128+p]; bf16 (host upcasts) — halves the
    # eviction DVE time and the output DMA bytes, ~0.17% added rounding.
    out = nc.dram_tensor("out", [out_f, tok_per_core], mybir.dt.bfloat16, kind="ExternalOutput")

    xt_ap = xt.ap().rearrange("p (ko t) -> p ko t", ko=KO)
    wt_ap = wt.ap().rearrange("(nb p) (ko o) -> p nb ko o", p=P, o=P)
    out_ap = out.ap().rearrange("(nb p) t -> p nb t", p=P)

    with tile.TileContext(nc) as tc:
        with (
            tc.tile_pool(name="xt_pool", bufs=1) as xt_pool,
            tc.tile_pool(name="warm_pool", bufs=2) as warm_pool,
            tc.tile_pool(name="bias_pool", bufs=1) as bias_pool,
            tc.tile_pool(name="wt_pool", bufs=6) as wt_pool,
            tc.tile_pool(name="out_pool", bufs=4) as out_pool,
            tc.tile_pool(name="psum", bufs=8, space="PSUM") as psum_pool,
        ):
            xt_sb = xt_pool.tile([P, KO, tok_per_core], mybir.dt.bfloat16)
            bias_sb = bias_pool.tile([P, NB], mybir.dt.float32)

            wt_tiles = {}

            def wt_tile(o):
                if o not in wt_tiles:
                    wt_tiles[o] = wt_pool.tile(
                        [P, KO, P], mybir.dt.bfloat16, name=f"wt_{o}", tag="wt"
                    )
                return wt_tiles[o]

            def load_wt(o, kb, kbe):
                nc.sync.dma_start(wt_tile(o)[:, kb:kbe, :], wt_ap[:, o, kb:kbe, :])

            def load_xt(kb, kbe):
                nc.sync.dma_start(xt_sb[:, kb:kbe, :], xt_ap[:, kb:kbe, :])

            def evict(o, ps, cb, cbe):
                ot = out_pool.tile(
                    [P, cbe - cb], mybir.dt.bfloat16, name=f"ot_{o}_{cb}", tag="ot"
                )
                nc.vector.tensor_scalar_add(ot[:], ps[:], bias_sb[:, o : o + 1])
                nc.sync.dma_start(out_ap[:, o, cb:cbe], ot[:])

            def evict_pair(o, ps0, ps1):
                # One [P, 1024] output DMA per o-tile instead of two halves:
                # fewer DMA instructions -> fewer semaphores -> shorter
                # end-of-NEFF semaphore-reset epilogue.
                ot = out_pool.tile([P, 1024], mybir.dt.bfloat16, name=f"ot_{o}", tag="ot")
                nc.vector.tensor_scalar_add(ot[:, 0:512], ps0[:], bias_sb[:, o : o + 1])
                nc.vector.tensor_scalar_add(ot[:, 512:1024], ps1[:], bias_sb[:, o : o + 1])
                nc.sync.dma_start(out_ap[:, o, :], ot[:])

            # ---- PE pre-warm: dummy matmuls on scratch during the DMA
            # startup hole so HAM un-throttles (K=8/8) before the first real
            # matmul instead of ~5us into phase A.
            warm_x = warm_pool.tile([P, 512], mybir.dt.bfloat16, name="warm_x")
            warm_w = warm_pool.tile([P, P], mybir.dt.bfloat16, name="warm_w")
            nc.vector.memset(warm_x[:], 0.0)
            nc.vector.memset(warm_w[:], 0.0)
            warm_ps = psum_pool.tile([P, 512], mybir.dt.float32, name="warm_ps", tag="ps")
            for _ in range(10):
                nc.tensor.matmul(
                    warm_ps[:], lhsT=warm_w[:], rhs=warm_x[:], start=True, stop=True
                )

            # ---- Phase A: out-tiles 0..3, k-outer so DMA demand is smooth ----
            # chunks sized fine at the start so the first matmul gates on
            # ~0.3 MiB of DMA, coarser later. xt rides finer-grained DMA
            # pieces than the MM chunking so matmuls gate on small arrivals.
            chunks = [(0, 1), (1, 4), (4, 8), (8, 16), (16, 24), (24, 32)]
            xt_pieces = [(0, 1), (1, 2), (2, 4), (4, 6), (6, 8)] + [
                (b, b + 4) for b in range(8, KO, 4)
            ]
            ps_a = {
                (o, h): psum_pool.tile([P, 512], mybir.dt.float32, name=f"psA_{o}_{h}", tag="ps")
                for o in range(A_TILES)
                for h in range(NH)
            }
            for ci, (kb, kbe) in enumerate(chunks):
                for pb, pbe in xt_pieces:
                    if pb >= kb and pbe <= kbe:
                        load_xt(pb, pbe)
                for o in range(A_TILES):
                    load_wt(o, kb, kbe)
                if ci == 2:
                    nc.sync.dma_start(bias_sb[:], biasr.ap())
                if ci == len(chunks) - 2:
                    load_wt(A_TILES, 0, KO)  # phase-B prefetch into spare bufs
                if ci == len(chunks) - 1:
                    load_wt(A_TILES + 1, 0, KO)
            for kb, kbe in chunks:
                for o in range(A_TILES):
                    wto = wt_tile(o)
                    for ko in range(kb, kbe):
                        for h in range(NH):
                            nc.tensor.matmul(
                                ps_a[(o, h)][:],
                                lhsT=wto[:, ko, :],
                                rhs=xt_sb[:, ko, h * 512 : (h + 1) * 512],
                                start=(ko == 0),
                                stop=(ko == KO - 1),
                            )
            for o in range(A_TILES):
                evict_pair(o, ps_a[(o, 0)], ps_a[(o, 1)])

            # ---- Phase B: o-major merged k-sweeps (64 MMs) at pure PE rate.
            # The last o-tile runs four sequential 256-column quarter-sweeps
            # so only one small eviction + DMA trails the final matmul.
            for o in range(A_TILES, NB):
                if o + 2 < NB:
                    load_wt(o + 2, 0, KO)
                wto = wt_tile(o)
                if o == NB - 1:
                    for q in range(4):
                        ps = psum_pool.tile(
                            [P, 256], mybir.dt.float32, name=f"ps_{o}_q{q}", tag="ps"
                        )
                        for ko in range(KO):
                            nc.tensor.matmul(
                                ps[:],
                                lhsT=wto[:, ko, :],
                                rhs=xt_sb[:, ko, q * 256 : (q + 1) * 256],
                                start=(ko == 0),
                                stop=(ko == KO - 1),
                            )
                        evict(o, ps, q * 256, (q + 1) * 256)
                    continue
                ps = {
                    h: psum_pool.tile(
                        [P, 512], mybir.dt.float32, name=f"ps_{o}_{h}", tag="ps"
                    )
                    for h in range(NH)
                }
                for ko in range(KO):
                    for h in range(NH):
                        nc.tensor.matmul(
                            ps[h][:],
                            lhsT=wto[:, ko, :],
                            rhs=xt_sb[:, ko, h * 512 : (h + 1) * 512],
                            start=(ko == 0),
                            stop=(ko == KO - 1),
                        )
                evict_pair(o, ps[0], ps[1])

    nc.compile()
    _CACHE[key] = nc
    return nc


def _densify_a(values, row_ids, col_ids, in_f=IN_F, out_f=OUT_F):
    """A[i, o] = sum of values[k] over k with col_ids[k]==i, row_ids[k]==o."""
    idx = col_ids.astype(np.int64) * out_f + row_ids.astype(np.int64)
    a = np.bincount(idx, weights=values.astype(np.float64), minlength=in_f * out_f)
    return a.astype(np.float32).reshape(in_f, out_f)


def kernel(x, values, row_ids, col_ids, bias):
    from concourse import bass_utils

    if os.environ.get("BASS_TRACE"):
        _ensure_ntff_hook()
        _patch_upload()

    nc = build_program()

    x = np.asarray(x, dtype=np.float32)
    values = np.asarray(values, dtype=np.float32)
    row_ids = np.asarray(row_ids)
    col_ids = np.asarray(col_ids)
    bias = np.asarray(bias, dtype=np.float32)

    KO = IN_F // P
    NB = OUT_F // P
    tpc = TOKENS // N_CORES

    a = _densify_a(values, row_ids, col_ids)  # [in_f, out_f] f32
    # wt2[nb, p, ko, o] = A[ko*128+p, nb*128+o]
    wt2 = np.ascontiguousarray(
        a.reshape(KO, P, NB, P).transpose(2, 1, 0, 3).reshape(OUT_F, IN_F)
    ).astype(ml_dtypes.bfloat16)
    bias2 = np.ascontiguousarray(bias.reshape(NB, P).T).astype(np.float32)

    in_maps = []
    for c in range(N_CORES):
        xT = x[c * tpc : (c + 1) * tpc, :].T  # [in_f, tpc]
        xt2 = np.ascontiguousarray(
            xT.reshape(KO, P, tpc).transpose(1, 0, 2).reshape(P, KO * tpc)
        ).astype(ml_dtypes.bfloat16)
        in_maps.append({"xt": xt2, "wt": wt2, "biasr": bias2})

    res = bass_utils.run_bass_kernel_spmd(nc, in_maps, core_ids=list(range(N_CORES)))
    global last_results
    last_results = res
    return np.ascontiguousarray(
        np.concatenate(
            [res.results[c]["out"].T.astype(np.float32) for c in range(N_CORES)],
            axis=0,
        )
    )


last_results = None


# revision 28
# speedup vs baseline: 1.0067x; 1.0067x over previous
"""CSR Linear kernel for TRN2: out = x @ W^T + bias, W from COO nonzeros.

Strategy: data-parallel over tokens across 8 NeuronCores. Host densifies the
sparse weight into A[in, out] (duplicate coords summed) in bf16; each core
computes its 1024-token shard as out^T = A^T-tiles stationary on the PE with
x^T streaming:  psum[128 outf, 512 tok] += A_tile[128 k, 128 outf].T @
xT[128 k, 512 tok].  With out-features on PSUM partitions the bias add is a
per-partition tensor_scalar on the eviction. bf16 operands halve DMA bytes
and enable fast weight load; phase A runs the first 4 out-tiles k-outer so
DMA demand stays under the per-core HBM rate from the first matmul, phase B
runs o-major k-sweeps at pure PE rate with per-sweep evictions.
"""

import os
import sys
import types

import ml_dtypes
import numpy as np

TOKENS = 8192
IN_F = 4096
OUT_F = 4096
N_CORES = 8
P = 128

_CACHE = {}


def _ensure_ntff_hook():
    """Register the axon NTFF profile hook if the antenv stub lacks it.

    Only needed when tracing (BASS_TRACE=1); harmless otherwise. In
    environments with a real antenv.axon_hooks this is a no-op.
    """
    try:
        import antenv.axon_hooks  # noqa: F401

        return
    except ImportError:
        pass
    try:
        import antenv
        from trn_agent_boot.trn_boot import _ntff_profile_via_ctypes

        hooks = types.ModuleType("antenv.axon_hooks")
        hooks._hook = _ntff_profile_via_ctypes("/opt/axon/libaxon_pjrt.so")
        hooks.set_axon_ntff_profile_hook = lambda h: setattr(hooks, "_hook", h)
        hooks.get_axon_ntff_profile_hook = lambda: hooks._hook
        sys.modules["antenv.axon_hooks"] = hooks
        antenv.axon_hooks = hooks
    except Exception:
        pass


def _patch_upload():
    """Make trace artifact upload fall back to the local tmpdir when no
    artifact bucket is reachable (container environments)."""
    from concourse import bass_utils

    orig = bass_utils.upload_artifacts
    if getattr(orig, "_kernel_patched", False):
        return

    def _safe_upload(tmpdir):
        try:
            return orig(tmpdir)
        except Exception:
            return tmpdir

    _safe_upload._kernel_patched = True
    bass_utils.upload_artifacts = _safe_upload


def build_program(tok_per_core=TOKENS // N_CORES, in_f=IN_F, out_f=OUT_F):
    """Build + compile the per-core Bass program.

    outT[out_f, tok_per_core] = sum_k A[k, :].T-tiles @ xT[k, tokens] + bias
    with A [in_f, out_f] bf16 (host-densified W^T), xT [in_f, tok] bf16.
    """
    key = (tok_per_core, in_f, out_f)
    if key in _CACHE:
        return _CACHE[key]

    import concourse.bacc as bacc
    import concourse.mybir as mybir
    import concourse.tile as tile

    KO = in_f // P  # 32 contraction tiles
    NB = out_f // P  # 32 out-feature tiles
    NH = tok_per_core // 512  # 2 token halves (psum bank = 512 f32)
    A_TILES = 4  # phase-A out-tiles (k-outer), 4*NH = 8 psum banks

    nc = bacc.Bacc("TRN2", target_bir_lowering=False, debug=False)

    # xt2[p, ko*T + t] = x_shard^T[ko*128+p, t]
    xt = nc.dram_tensor("xt", [P, KO * tok_per_core], mybir.dt.bfloat16, kind="ExternalInput")
    # wt2[nb*128+p, ko*128+o] = A[ko*128+p, nb*128+o]
    wt = nc.dram_tensor("wt", [out_f, in_f], mybir.dt.bfloat16, kind="ExternalInput")
    # biasr[p, nb] = bias[nb*128+p]
    biasr = nc.dram_tensor("biasr", [P, NB], mybir.dt.float32, kind="ExternalInput")
    # outT[nb*128+p, t] = out[t, nb*128+p]; bf16 (host upcasts) — halves the
    # eviction DVE time and the output DMA bytes, ~0.17% added rounding.
    out = nc.dram_tensor("out", [out_f, tok_per_core], mybir.dt.bfloat16, kind="ExternalOutput")

    xt_ap = xt.ap().rearrange("p (ko t) -> p ko t", ko=KO)
    wt_ap = wt.ap().rearrange("(nb p) (ko o) -> p nb ko o", p=P, o=P)
    out_ap = out.ap().rearrange("(nb p) t -> p nb t", p=P)

    with tile.TileContext(nc) as tc:
        with (
            tc.tile_pool(name="xt_pool", bufs=1) as xt_pool,
            tc.tile_pool(name="warm_pool", bufs=2) as warm_pool,
            tc.tile_pool(name="bias_pool", bufs=1) as bias_pool,
            tc.tile_pool(name="wt_pool", bufs=6) as wt_pool,
            tc.tile_pool(name="out_pool", bufs=4) as out_pool,
            tc.tile_pool(name="psum", bufs=8, space="PSUM") as psum_pool,
        ):
            xt_sb = xt_pool.tile([P, KO, tok_per_core], mybir.dt.bfloat16)
            bias_sb = bias_pool.tile([P, NB], mybir.dt.float32)

            wt_tiles = {}

            def wt_tile(o):
                if o not in wt_tiles:
                    wt_tiles[o] = wt_pool.tile(
                        [P, KO, P], mybir.dt.bfloat16, name=f"wt_{o}", tag="wt"
                    )
                return wt_tiles[o]

            def load_wt(o, kb, kbe):
                nc.sync.dma_start(wt_tile(o)[:, kb:kbe, :], wt_ap[:, o, kb:kbe, :])

            def load_xt(kb, kbe):
                nc.sync.dma_start(xt_sb[:, kb:kbe, :], xt_ap[:, kb:kbe, :])

            def evict(o, ps, cb, cbe):
                ot = out_pool.tile(
                    [P, cbe - cb], mybir.dt.bfloat16, name=f"ot_{o}_{cb}", tag="ot"
                )
                nc.vector.tensor_scalar_add(ot[:], ps[:], bias_sb[:, o : o + 1])
                nc.sync.dma_start(out_ap[:, o, cb:cbe], ot[:])



            # ---- PE pre-warm: dummy matmuls on scratch during the DMA
            # startup hole so HAM un-throttles (K=8/8) before the first real
            # matmul instead of ~5us into phase A.
            warm_x = warm_pool.tile([P, 512], mybir.dt.bfloat16, name="warm_x")
            warm_w = warm_pool.tile([P, P], mybir.dt.bfloat16, name="warm_w")
            nc.vector.memset(warm_x[:], 0.0)
            nc.vector.memset(warm_w[:], 0.0)
            warm_ps = psum_pool.tile([P, 512], mybir.dt.float32, name="warm_ps", tag="ps")
            for _ in range(10):
                nc.tensor.matmul(
                    warm_ps[:], lhsT=warm_w[:], rhs=warm_x[:], start=True, stop=True
                )

            # ---- Phase A: out-tiles 0..3, k-outer so DMA demand is smooth ----
            # chunks sized fine at the start so the first matmul gates on
            # ~0.3 MiB of DMA, coarser later. xt rides finer-grained DMA
            # pieces than the MM chunking so matmuls gate on small arrivals.
            chunks = [(0, 1), (1, 4), (4, 8), (8, 16), (16, 24), (24, 32)]
            xt_pieces = [(0, 1), (1, 2), (2, 4), (4, 6), (6, 8)] + [
                (b, b + 2) for b in range(8, KO, 2)
            ]
            ps_a = {
                (o, h): psum_pool.tile([P, 512], mybir.dt.float32, name=f"psA_{o}_{h}", tag="ps")
                for o in range(A_TILES)
                for h in range(NH)
            }
            for ci, (kb, kbe) in enumerate(chunks):
                for pb, pbe in xt_pieces:
                    if pb >= kb and pbe <= kbe:
                        load_xt(pb, pbe)
                for o in range(A_TILES):
                    load_wt(o, kb, kbe)
                if ci == 2:
                    nc.sync.dma_start(bias_sb[:], biasr.ap())
                if ci == len(chunks) - 2:
                    load_wt(A_TILES, 0, KO)  # phase-B prefetch into spare bufs
                if ci == len(chunks) - 1:
                    load_wt(A_TILES + 1, 0, KO)
            for kb, kbe in chunks:
                for o in range(A_TILES):
                    wto = wt_tile(o)
                    for ko in range(kb, kbe):
                        for h in range(NH):
                            nc.tensor.matmul(
                                ps_a[(o, h)][:],
                                lhsT=wto[:, ko, :],
                                rhs=xt_sb[:, ko, h * 512 : (h + 1) * 512],
                                start=(ko == 0),
                                stop=(ko == KO - 1),
                            )
            for o in range(A_TILES):
                for h in range(NH):
                    evict(o, ps_a[(o, h)], h * 512, (h + 1) * 512)

            # ---- Phase B: o-major merged k-sweeps (64 MMs) at pure PE rate.
            # The last o-tile runs four sequential 256-column quarter-sweeps
            # so only one small eviction + DMA trails the final matmul.
            for o in range(A_TILES, NB):
                if o + 2 < NB:
                    load_wt(o + 2, 0, KO)
                wto = wt_tile(o)
                if o == NB - 1:
                    for q in range(4):
                        ps = psum_pool.tile(
                            [P, 256], mybir.dt.float32, name=f"ps_{o}_q{q}", tag="ps"
                        )
                        for ko in range(KO):
                            nc.tensor.matmul(
                                ps[:],
                                lhsT=wto[:, ko, :],
                                rhs=xt_sb[:, ko, q * 256 : (q + 1) * 256],
                                start=(ko == 0),
                                stop=(ko == KO - 1),
                            )
                        evict(o, ps, q * 256, (q + 1) * 256)
                    continue
                ps = {
                    h: psum_pool.tile(
                        [P, 512], mybir.dt.float32, name=f"ps_{o}_{h}", tag="ps"
                    )
                    for h in range(NH)
                }
                for ko in range(KO):
                    for h in range(NH):
                        nc.tensor.matmul(
                            ps[h][:],
                            lhsT=wto[:, ko, :],
                            rhs=xt_sb[:, ko, h * 512 : (h + 1) * 512],
                            start=(ko == 0),
                            stop=(ko == KO - 1),
                        )
                for h in range(NH):
                    evict(o, ps[h], h * 512, (h + 1) * 512)

    nc.compile()
    _CACHE[key] = nc
    return nc


def _densify_a(values, row_ids, col_ids, in_f=IN_F, out_f=OUT_F):
    """A[i, o] = sum of values[k] over k with col_ids[k]==i, row_ids[k]==o."""
    idx = col_ids.astype(np.int64) * out_f + row_ids.astype(np.int64)
    a = np.bincount(idx, weights=values.astype(np.float64), minlength=in_f * out_f)
    return a.astype(np.float32).reshape(in_f, out_f)


def kernel(x, values, row_ids, col_ids, bias):
    from concourse import bass_utils

    if os.environ.get("BASS_TRACE"):
        _ensure_ntff_hook()
        _patch_upload()

    nc = build_program()

    x = np.asarray(x, dtype=np.float32)
    values = np.asarray(values, dtype=np.float32)
    row_ids = np.asarray(row_ids)
    col_ids = np.asarray(col_ids)
    bias = np.asarray(bias, dtype=np.float32)

    KO = IN_F // P
    NB = OUT_F // P
    tpc = TOKENS // N_CORES

    a = _densify_a(values, row_ids, col_ids)  # [in_f, out_f] f32
    # wt2[nb, p, ko, o] = A[ko*128+p, nb*128+o]
    wt2 = np.ascontiguousarray(
        a.reshape(KO, P, NB, P).transpose(2, 1, 0, 3).reshape(OUT_F, IN_F)
    ).astype(ml_dtypes.bfloat16)
    bias2 = np.ascontiguousarray(bias.reshape(NB, P).T).astype(np.float32)

    in_maps = []
    for c in range(N_CORES):
        xT = x[c * tpc : (c + 1) * tpc, :].T  # [in_f, tpc]
        xt2 = np.ascontiguousarray(
            xT.reshape(KO, P, tpc).transpose(1, 0, 2).reshape(P, KO * tpc)
        ).astype(ml_dtypes.bfloat16)
        in_maps.append({"xt": xt2, "wt": wt2, "biasr": bias2})

    res = bass_utils.run_bass_kernel_spmd(nc, in_maps, core_ids=list(range(N_CORES)))
    global last_results
    last_results = res
    return np.ascontiguousarray(
        np.concatenate(
            [res.results[c]["out"].T.astype(np.float32) for c in range(N_CORES)],
            axis=0,
        )
    )


last_results = None


# revision 32
# speedup vs baseline: 1.0098x; 1.0030x over previous
"""CSR Linear kernel for TRN2: out = x @ W^T + bias, W from COO nonzeros.

Strategy: data-parallel over tokens across 8 NeuronCores. Host densifies the
sparse weight into A[in, out] (duplicate coords summed) in bf16; each core
computes its 1024-token shard as out^T = A^T-tiles stationary on the PE with
x^T streaming:  psum[128 outf, 512 tok] += A_tile[128 k, 128 outf].T @
xT[128 k, 512 tok].  With out-features on PSUM partitions the bias add is a
per-partition tensor_scalar on the eviction. bf16 operands halve DMA bytes
and enable fast weight load; phase A runs the first 4 out-tiles k-outer so
DMA demand stays under the per-core HBM rate from the first matmul, phase B
runs o-major k-sweeps at pure PE rate with per-sweep evictions.
"""

import os
import sys
import types

import ml_dtypes
import numpy as np

TOKENS = 8192
IN_F = 4096
OUT_F = 4096
N_CORES = 8
P = 128

_CACHE = {}


def _ensure_ntff_hook():
    """Register the axon NTFF profile hook if the antenv stub lacks it.

    Only needed when tracing (BASS_TRACE=1); harmless otherwise. In
    environments with a real antenv.axon_hooks this is a no-op.
    """
    try:
        import antenv.axon_hooks  # noqa: F401

        return
    except ImportError:
        pass
    try:
        import antenv
        from trn_agent_boot.trn_boot import _ntff_profile_via_ctypes

        hooks = types.ModuleType("antenv.axon_hooks")
        hooks._hook = _ntff_profile_via_ctypes("/opt/axon/libaxon_pjrt.so")
        hooks.set_axon_ntff_profile_hook = lambda h: setattr(hooks, "_hook", h)
        hooks.get_axon_ntff_profile_hook = lambda: hooks._hook
        sys.modules["antenv.axon_hooks"] = hooks
        antenv.axon_hooks = hooks
    except Exception:
        pass


def _patch_upload():
    """Make trace artifact upload fall back to the local tmpdir when no
    artifact bucket is reachable (container environments)."""
    from concourse import bass_utils

    orig = bass_utils.upload_artifacts
    if getattr(orig, "_kernel_patched", False):
        return

    def _safe_upload(tmpdir):
        try:
            return orig(tmpdir)
        except Exception:
            return tmpdir

    _safe_upload._kernel_patched = True
    bass_utils.upload_artifacts = _safe_upload


def build_program(tok_per_core=TOKENS // N_CORES, in_f=IN_F, out_f=OUT_F):
    """Build + compile the per-core Bass program.

    outT[out_f, tok_per_core] = sum_k A[k, :].T-tiles @ xT[k, tokens] + bias
    with A [in_f, out_f] bf16 (host-densified W^T), xT [in_f, tok] bf16.
    """
    key = (tok_per_core, in_f, out_f)
    if key in _CACHE:
        return _CACHE[key]

    import concourse.bacc as bacc
    import concourse.mybir as mybir
    import concourse.tile as tile

    KO = in_f // P  # 32 contraction tiles
    NB = out_f // P  # 32 out-feature tiles
    NH = tok_per_core // 512  # 2 token halves (psum bank = 512 f32)
    A_TILES = 4  # phase-A out-tiles (k-outer), 4*NH = 8 psum banks

    nc = bacc.Bacc("TRN2", target_bir_lowering=False, debug=False)
    # All DMAs in this kernel issue from the sync engine; drop the unused
    # scalar-engine HWDGE queue family so the NEFF declares (and tears down)
    # 16 fewer queues at exit.
    nc.m.queues = [q for q in nc.m.queues if q.name != "qScalarDynamicHW"]

    # xt2[p, ko*T + t] = x_shard^T[ko*128+p, t]
    xt = nc.dram_tensor("xt", [P, KO * tok_per_core], mybir.dt.bfloat16, kind="ExternalInput")
    # wt2[nb*128+p, ko*128+o] = A[ko*128+p, nb*128+o]
    wt = nc.dram_tensor("wt", [out_f, in_f], mybir.dt.bfloat16, kind="ExternalInput")
    # biasr[p, nb] = bias[nb*128+p]
    biasr = nc.dram_tensor("biasr", [P, NB], mybir.dt.float32, kind="ExternalInput")
    # outT[nb*128+p, t] = out[t, nb*128+p]; bf16 (host upcasts) — halves the
    # eviction DVE time and the output DMA bytes, ~0.17% added rounding.
    out = nc.dram_tensor("out", [out_f, tok_per_core], mybir.dt.bfloat16, kind="ExternalOutput")

    xt_ap = xt.ap().rearrange("p (ko t) -> p ko t", ko=KO)
    wt_ap = wt.ap().rearrange("(nb p) (ko o) -> p nb ko o", p=P, o=P)
    out_ap = out.ap().rearrange("(nb p) t -> p nb t", p=P)

    with tile.TileContext(nc) as tc:
        with (
            tc.tile_pool(name="xt_pool", bufs=1) as xt_pool,
            tc.tile_pool(name="warm_pool", bufs=1) as warm_pool,
            tc.tile_pool(name="bias_pool", bufs=1) as bias_pool,
            tc.tile_pool(name="wt_pool", bufs=5) as wt_pool,
            tc.tile_pool(name="out_pool", bufs=4) as out_pool,
            tc.tile_pool(name="psum", bufs=8, space="PSUM") as psum_pool,
        ):
            xt_sb = xt_pool.tile([P, KO, tok_per_core], mybir.dt.bfloat16)
            bias_sb = bias_pool.tile([P, NB], mybir.dt.float32)

            wt_tiles = {}

            def wt_tile(o):
                if o not in wt_tiles:
                    wt_tiles[o] = wt_pool.tile(
                        [P, KO, P], mybir.dt.bfloat16, name=f"wt_{o}", tag="wt"
                    )
                return wt_tiles[o]

            def load_wt(o, kb, kbe):
                nc.sync.dma_start(wt_tile(o)[:, kb:kbe, :], wt_ap[:, o, kb:kbe, :])

            def load_xt(kb, kbe):
                nc.sync.dma_start(xt_sb[:, kb:kbe, :], xt_ap[:, kb:kbe, :])

            def evict(o, ps, cb, cbe):
                ot = out_pool.tile(
                    [P, cbe - cb], mybir.dt.bfloat16, name=f"ot_{o}_{cb}", tag="ot"
                )
                nc.vector.tensor_scalar_add(ot[:], ps[:], bias_sb[:, o : o + 1])
                nc.sync.dma_start(out_ap[:, o, cb:cbe], ot[:])



            # ---- PE pre-warm: dummy matmuls on scratch during the DMA
            # startup hole so HAM un-throttles (K=8/8) before the first real
            # matmul instead of ~5us into phase A.
            warm_x = warm_pool.tile([P, 512], mybir.dt.bfloat16, name="warm_x")
            nc.vector.memset(warm_x[:], 0.0)
            warm_ps = psum_pool.tile([P, 512], mybir.dt.float32, name="warm_ps", tag="ps")
            for _ in range(6):
                nc.tensor.matmul(
                    warm_ps[:], lhsT=warm_x[:, 0:P], rhs=warm_x[:], start=True, stop=True
                )

            # ---- Phase A: out-tiles 0..3, k-outer so DMA demand is smooth ----
            # chunks sized fine at the start so the first matmul gates on
            # ~0.3 MiB of DMA, coarser later. xt rides finer-grained DMA
            # pieces than the MM chunking so matmuls gate on small arrivals.
            chunks = [(0, 1), (1, 4), (4, 8), (8, 16), (16, 24), (24, 32)]
            xt_pieces = [(0, 1), (1, 2), (2, 4), (4, 6), (6, 8)] + [
                (b, b + 2) for b in range(8, KO, 2)
            ]
            ps_a = {
                (o, h): psum_pool.tile([P, 512], mybir.dt.float32, name=f"psA_{o}_{h}", tag="ps")
                for o in range(A_TILES)
                for h in range(NH)
            }
            for ci, (kb, kbe) in enumerate(chunks):
                for pb, pbe in xt_pieces:
                    if pb >= kb and pbe <= kbe:
                        load_xt(pb, pbe)
                for o in range(A_TILES):
                    load_wt(o, kb, kbe)
                if ci == 2:
                    nc.sync.dma_start(bias_sb[:], biasr.ap())
                if ci == len(chunks) - 2:
                    load_wt(A_TILES, 0, KO)  # phase-B prefetch into spare bufs
                if ci == len(chunks) - 1:
                    load_wt(A_TILES + 1, 0, KO)
            for kb, kbe in chunks:
                for o in range(A_TILES):
                    wto = wt_tile(o)
                    for ko in range(kb, kbe):
                        for h in range(NH):
                            nc.tensor.matmul(
                                ps_a[(o, h)][:],
                                lhsT=wto[:, ko, :],
                                rhs=xt_sb[:, ko, h * 512 : (h + 1) * 512],
                                start=(ko == 0),
                                stop=(ko == KO - 1),
                            )
            for o in range(A_TILES):
                for h in range(NH):
                    evict(o, ps_a[(o, h)], h * 512, (h + 1) * 512)

            # ---- Phase B: o-major merged k-sweeps (64 MMs) at pure PE rate.
            # The last o-tile runs four sequential 256-column quarter-sweeps
            # so only one small eviction + DMA trails the final matmul.
            for o in range(A_TILES, NB):
                if o + 2 < NB:
                    load_wt(o + 2, 0, KO)
                wto = wt_tile(o)
                if o == NB - 1:
                    for q in range(4):
                        ps = psum_pool.tile(
                            [P, 256], mybir.dt.float32, name=f"ps_{o}_q{q}", tag="ps"
                        )
                        for ko in range(KO):
                            nc.tensor.matmul(
                                ps[:],
                                lhsT=wto[:, ko, :],
                                rhs=xt_sb[:, ko, q * 256 : (q + 1) * 256],
                                start=(ko == 0),
                                stop=(ko == KO - 1),
                            )
                        evict(o, ps, q * 256, (q + 1) * 256)
                    continue
                ps = {
                    h: psum_pool.tile(
                        [P, 512], mybir.dt.float32, name=f"ps_{o}_{h}", tag="ps"
                    )
                    for h in range(NH)
                }
                for ko in range(KO):
                    for h in range(NH):
                        nc.tensor.matmul(
                            ps[h][:],
                            lhsT=wto[:, ko, :],
                            rhs=xt_sb[:, ko, h * 512 : (h + 1) * 512],
                            start=(ko == 0),
                            stop=(ko == KO - 1),
                        )
                for h in range(NH):
                    evict(o, ps[h], h * 512, (h + 1) * 512)

    nc.compile()
    _CACHE[key] = nc
    return nc


def _densify_a(values, row_ids, col_ids, in_f=IN_F, out_f=OUT_F):
    """A[i, o] = sum of values[k] over k with col_ids[k]==i, row_ids[k]==o."""
    idx = col_ids.astype(np.int64) * out_f + row_ids.astype(np.int64)
    a = np.bincount(idx, weights=values.astype(np.float64), minlength=in_f * out_f)
    return a.astype(np.float32).reshape(in_f, out_f)


def kernel(x, values, row_ids, col_ids, bias):
    from concourse import bass_utils

    if os.environ.get("BASS_TRACE"):
        _ensure_ntff_hook()
        _patch_upload()

    nc = build_program()

    x = np.asarray(x, dtype=np.float32)
    values = np.asarray(values, dtype=np.float32)
    row_ids = np.asarray(row_ids)
    col_ids = np.asarray(col_ids)
    bias = np.asarray(bias, dtype=np.float32)

    KO = IN_F // P
    NB = OUT_F // P
    tpc = TOKENS // N_CORES

    a = _densify_a(values, row_ids, col_ids)  # [in_f, out_f] f32
    # wt2[nb, p, ko, o] = A[ko*128+p, nb*128+o]
    wt2 = np.ascontiguousarray(
        a.reshape(KO, P, NB, P).transpose(2, 1, 0, 3).reshape(OUT_F, IN_F)
    ).astype(ml_dtypes.bfloat16)
    bias2 = np.ascontiguousarray(bias.reshape(NB, P).T).astype(np.float32)

    in_maps = []
    for c in range(N_CORES):
        xT = x[c * tpc : (c + 1) * tpc, :].T  # [in_f, tpc]
        xt2 = np.ascontiguousarray(
            xT.reshape(KO, P, tpc).transpose(1, 0, 2).reshape(P, KO * tpc)
        ).astype(ml_dtypes.bfloat16)
        in_maps.append({"xt": xt2, "wt": wt2, "biasr": bias2})

    res = bass_utils.run_bass_kernel_spmd(nc, in_maps, core_ids=list(range(N_CORES)))
    global last_results
    last_results = res
    return np.ascontiguousarray(
        np.concatenate(
            [res.results[c]["out"].T.astype(np.float32) for c in range(N_CORES)],
            axis=0,
        )
    )


last_results = None
